# revision 3
# baseline (speedup 1.0000x reference)
"""Trainium2 Bass kernel for ConvTemporalGraphical (gnn_message_passing).

Reference computation (fp32):
    y   = einsum('nctv,oc->notv', x, W) + b        # 1x1 conv channel mix
    out = einsum('nkctv,kvw->nctw', y.reshape(n,K,C,t,v), A)

Shapes: x [16,128,256,64] f32, A [3,64,64], W [384,128], b [384].

Strategy (8 NeuronCores, data-parallel over N, 2 samples per core):
  The two contractions are reordered as
      Z_k[ci,t,w] = sum_v x[ci,t,v] * A[k,v,w]          (graph mixing first)
      out[c,t,w]  = sum_k sum_ci W[(k,c),ci] * Z_k[ci,t,w] + bias2[c,w]
  with bias2[c,w] = sum_{k,v} b[(k,c)] A[k,v,w] (host-precomputed).

  All matmul operands are bf16 (validated ~4e-3 max rel err vs the 2e-2
  gate): bf16 stationaries let LDWEIGHTS overlap in-flight matmuls
  (fp32/fp32r force a fused, non-overlappable weight load per matmul),
  transposes run at 1 cyc/row, and bf16 PSUM reads drain at 2x.

  On-device per (n, 32-t chunk):
    1. gpsimd cast-DMA x chunk [ci=128, 32*64] f32->bf16 (1MB HBM read
       per transfer; the SDMA CME converts inline, so the cast is free).
    2. PE-transpose per 2-t pair: [ci,128] -> xt [(t0 v|t1 v), ci], bf16.
       Four transposes share one PSUM tile -> one [128,512] 2x-rate drain.
    3. Step A matmul (FD=384): lhsT=xt pair, rhs=MA where MA [128,384]
       is block-diag([Acat, Acat]), Acat[v,(k w)]=A[k,v,w]. Accumulate Z
       into a [ci, 32, 3, 64] bf16 SBUF buffer (drain casts f32->bf16).
    4. Step B matmul (FD=512): per 8-t group, accumulate over k in
       PSUM: lhsT=Wt[:,k,:] ([ci,c] bf16), rhs=Z[:, g8, k, :] (strided).
    5. Drain with fused bias add (DVE) -> out tile [c, 32, 64] f32 ->
       DMA out per chunk (1MB), alternating the sync/scalar HWDGE queues
       so the in (SWDGE) and out (HWDGE x2) streams keep all 16 SDMA
       engines fed.

kernel(**inputs) shards on host, runs the SPMD program on cores 0-7, and
concatenates the per-core outputs.
"""

import numpy as np

import concourse.bass as bass
import concourse.mybir as mybir
from concourse import bacc
from concourse.bass_utils import run_bass_kernel_spmd
from concourse.tile import TileContext

F32 = mybir.dt.float32
BF16 = mybir.dt.bfloat16

N, C_IN, C_OUT, K, T, V = 16, 128, 128, 3, 256, 64
N_CORES = 8
N_PER_CORE = N // N_CORES  # 2
TC = 32                    # t-chunk size
N_CHUNKS = T // TC         # 8
QG = TC // 8               # 4 groups (8 t's = 4 pairs) per chunk


def build(reps: int = 1):
    nc = bacc.Bacc(
        "TRN2", target_bir_lowering=False, debug=False, num_devices=N_CORES
    )
    xs = nc.dram_tensor("xs", [N_PER_CORE, C_IN, T, V], F32, kind="ExternalInput")
    wt = nc.dram_tensor("wt", [C_IN, K, C_OUT], BF16, kind="ExternalInput")
    ma = nc.dram_tensor("ma", [128, 2, K, V], BF16, kind="ExternalInput")
    bias2r = nc.dram_tensor("bias2r", [C_OUT, 8, V], F32, kind="ExternalInput")
    ident = nc.dram_tensor("ident", [128, 128], BF16, kind="ExternalInput")
    out = nc.dram_tensor(
        "out", [N_PER_CORE, C_OUT, T, V], F32, kind="ExternalOutput"
    )

    with TileContext(nc) as tc:
        with (
            tc.tile_pool(name="const", bufs=1) as cpool,
            tc.tile_pool(name="xin", bufs=3) as xpool,
            tc.tile_pool(name="xt", bufs=3) as xtpool,
            tc.tile_pool(name="z", bufs=2) as zpool,
            tc.tile_pool(name="o", bufs=3) as opool,
            tc.tile_pool(name="ps_xt", bufs=2, space="PSUM") as ps_xt,
            tc.tile_pool(name="ps_z", bufs=2, space="PSUM") as ps_z,
            tc.tile_pool(name="ps_o", bufs=2, space="PSUM") as ps_o,
        ):
            # consts ride the gpsimd DMA queue ahead of the first x chunk
            ident_sb = cpool.tile([128, 128], BF16, tag="ident")
            nc.gpsimd.dma_start(out=ident_sb[:], in_=ident[:])
            wt_sb = cpool.tile([C_IN, K, C_OUT], BF16, tag="wt")
            nc.gpsimd.dma_start(out=wt_sb[:], in_=wt[:])
            ma_sb = cpool.tile([128, 2, K, V], BF16, tag="ma")
            nc.gpsimd.dma_start(out=ma_sb[:], in_=ma[:])
            bias_sb = cpool.tile([C_OUT, 8, V], F32, tag="bias")
            nc.gpsimd.dma_start(out=bias_sb[:], in_=bias2r[:])

            # Software-pipelined emission: transposes of group i, step A of
            # group i-1, step B of group i-2, so every PE op's producer
            # drain has a full group-time to land.
            for _ in range(reps):
                groups = [
                    (n, c, q)
                    for n in range(N_PER_CORE)
                    for c in range(N_CHUNKS)
                    for q in range(QG)
                ]
                st = {}  # (n, c) -> chunk state

                def chunk_state(n, c):
                    if (n, c) not in st:
                        x_sb = xpool.tile([C_IN, TC * V], BF16, tag="x", name="x_sb")
                        t0 = c * TC
                        # SWDGE cast-DMA: f32 HBM -> bf16 SBUF, 1MB read
                        nc.gpsimd.dma_start(
                            out=x_sb[:],
                            in_=xs[n, :, t0 : t0 + TC, :],
                        )
                        st[(n, c)] = {
                            "x": x_sb,
                            "z": zpool.tile(
                                [C_IN, TC, K, V], BF16, tag="z", name="z_sb"
                            ),
                            "o": opool.tile(
                                [C_OUT, TC, V], F32, tag="o", name="o_sb"
                            ),
                            "xt": {},
                        }
                    return st[(n, c)]

                def stage_tp(n, c, q):
                    s = chunk_state(n, c)
                    x_sb = s["x"]
                    # 4 transposes -> one PSUM bank as ONE accumulation group
                    xt_ps = ps_xt.tile([128, 4, 128], BF16, tag="xtp")
                    for j in range(4):
                        col = (4 * q + j) * 128
                        nc.tensor.matmul(
                            xt_ps[:, j, :],
                            x_sb[:, col : col + 128],
                            ident_sb[:],
                            is_transpose=True,
                            start=(j == 0),
                            stop=(j == 3),
                            skip_group_check=True,
                        )
                    xt_sb = xtpool.tile([128, 4, 128], BF16, tag="xt")
                    nc.any.tensor_copy(out=xt_sb[:], in_=xt_ps[:])
                    s["xt"][q] = xt_sb

                def stage_a(n, c, q):
                    s = chunk_state(n, c)
                    xt_sb = s["xt"].pop(q)
                    # 2 pair-matmuls into one 2-bank PSUM tile, one batched
                    # drain (casts f32 PSUM -> bf16 Z)
                    for h in range(2):
                        z_ps = ps_z.tile([C_IN, 2, 512], F32, tag="zp")
                        for jj in range(2):
                            nc.tensor.matmul(
                                z_ps[:, jj, 0 : 2 * K * V],
                                xt_sb[:, 2 * h + jj, :],
                                ma_sb[:],
                                start=True,
                                stop=True,
                            )
                        t0 = 8 * q + 4 * h
                        nc.any.tensor_copy(
                            out=s["z"][:, t0 : t0 + 4, :, :],
                            in_=z_ps[:, :, 0 : 2 * K * V],
                        )

                def stage_b(n, c, q):
                    s = chunk_state(n, c)
                    o_ps = ps_o.tile([C_OUT, 8, V], F32, tag="op")
                    for k in range(K):
                        nc.tensor.matmul(
                            o_ps[:],
                            wt_sb[:, k, :],
                            s["z"][:, 8 * q : 8 * (q + 1), k, :],
                            start=(k == 0),
                            stop=(k == K - 1),
                        )
                    nc.vector.tensor_add(
                        out=s["o"][:, 8 * q : 8 * (q + 1), :],
                        in0=o_ps[:],
                        in1=bias_sb[:],
                    )
                    if q == QG - 1:
                        # alternate the two HWDGE rings for the out stream
                        eng = nc.sync if c % 2 == 0 else nc.scalar
                        eng.dma_start(
                            out=out[n, :, c * TC : (c + 1) * TC, :],
                            in_=s["o"][:],
                        )
                        del st[(n, c)]

                for i in range(len(groups) + 2):
                    if i < len(groups):
                        stage_tp(*groups[i])
                    if 1 <= i < len(groups) + 1:
                        stage_a(*groups[i - 1])
                    if i >= 2:
                        stage_b(*groups[i - 2])

    nc.compile()
    return nc


def to_bf16(a):
    import ml_dtypes

    return np.asarray(a, np.float32).astype(ml_dtypes.bfloat16)


def prep_weights(A, W, b):
    A = np.asarray(A, np.float32)
    W = np.asarray(W, np.float32)
    b = np.asarray(b, np.float32)
    wt = np.ascontiguousarray(
        W.reshape(K, C_OUT, C_IN).transpose(2, 0, 1)
    )  # [ci, k, c]
    acat = np.ascontiguousarray(A.transpose(1, 0, 2))  # [v, k, w]
    ma = np.zeros((128, 2, K, V), np.float32)
    ma[0:64, 0] = acat
    ma[64:128, 1] = acat
    bias2 = np.einsum("kc,kw->cw", b.reshape(K, C_OUT), A.sum(axis=1))
    bias2r = np.ascontiguousarray(
        np.broadcast_to(bias2[:, None, :], (C_OUT, 8, V))
    ).astype(np.float32)
    ident = np.eye(128, dtype=np.float32)
    return to_bf16(wt), to_bf16(ma), bias2r, to_bf16(ident)


_NC_CACHE = {}


def get_nc(reps: int = 1):
    if reps not in _NC_CACHE:
        _NC_CACHE[reps] = build(reps)
    return _NC_CACHE[reps]


def make_in_maps(x, A, W, b):
    x = np.asarray(x, np.float32)
    wt, ma, bias2r, ident = prep_weights(A, W, b)
    return [
        {
            "xs": np.ascontiguousarray(x[i * N_PER_CORE : (i + 1) * N_PER_CORE]),
            "wt": wt,
            "ma": ma,
            "bias2r": bias2r,
            "ident": ident,
        }
        for i in range(N_CORES)
    ]


def run(x, A, W, b, reps: int = 1):
    nc = get_nc(reps)
    in_maps = make_in_maps(x, A, W, b)
    res = run_bass_kernel_spmd(nc, in_maps, list(range(N_CORES)))
    return np.concatenate(
        [np.asarray(res.results[i]["out"]) for i in range(N_CORES)], axis=0
    )


def kernel(x, A, W, b):
    return run(x, A, W, b, reps=1)


# revision 4
# speedup vs baseline: 1.1230x; 1.1230x over previous
"""Trainium2 Bass kernel for ConvTemporalGraphical (gnn_message_passing).

Reference computation (fp32):
    y   = einsum('nctv,oc->notv', x, W) + b        # 1x1 conv channel mix
    out = einsum('nkctv,kvw->nctw', y.reshape(n,K,C,t,v), A)

Shapes: x [16,128,256,64] f32, A [3,64,64], W [384,128], b [384].

Strategy (8 NeuronCores, data-parallel over N, 2 samples per core):
  The two contractions are reordered as
      Z_k[ci,t,w] = sum_v x[ci,t,v] * A[k,v,w]          (graph mixing first)
      out[c,t,w]  = sum_k sum_ci W[(k,c),ci] * Z_k[ci,t,w] + bias2[c,w]
  with bias2[c,w] = sum_{k,v} b[(k,c)] A[k,v,w] (host-precomputed).

  On-device per (n, 32-t chunk):
    1. DMA x chunk [ci=128, 32*64] (1MB per transfer, 8KB/partition
       descriptors) on the sync HWDGE queue; consts ride the scalar
       HWDGE queue so the first x transfer starts at t=0.
    2. PE-transpose per 2-t pair: [ci,128] -> xt [(t0 v|t1 v), ci], fp32r.
       Four transposes share one PSUM tile so the drain is one [128,512] copy.
    3. Step A matmul (fp32r, FD=384): lhsT=xt pair, rhs=MA where MA [128,384]
       is block-diag([Acat, Acat]), Acat[v,(k w)]=A[k,v,w]. The zero blocks
       keep the two t's of a pair independent while using all 128 partitions.
       Two pair-outputs share one PSUM tile; Z accumulates into a
       [ci, 32, 3, 64] SBUF buffer.
    4. Step B matmul (fp32r, FD=512): per 8-t group, accumulate over k in
       PSUM: lhsT=Wt[:,k,:] ([ci,c]), rhs=Z[:, g8, k, :] (strided).
    5. Drain with fused bias add (DVE) -> out tile [c, 64, 64] spanning two
       chunks -> one 2MB DMA out per chunk-pair on the gpsimd SWDGE queue
       (separate from the input stream so both run concurrently).

  A short burst of dummy ident-transposes at t~1.5us (while the first x
  chunk is still in flight) trips the PE_HAM activity monitor early, so
  the PE clock gate reaches full rate sooner (it otherwise runs the
  first ~14us of real matmuls at half clock).

  fp32r (tf32-like) matmuls run at 1 cycle/column for FD>=256 with ~1.6e-4
  relative rounding error; the PE rounds operands internally so DMA/copy
  producers don't need explicit rounding passes. Transposing in fp32r is
  exact w.r.t. the final result: it pre-rounds x exactly as step A would.

kernel(**inputs) shards on host, runs the SPMD program on cores 0-7, and
concatenates the per-core outputs.
"""

import numpy as np

import concourse.bass as bass
import concourse.mybir as mybir
from concourse import bacc
from concourse.bass_utils import run_bass_kernel_spmd
from concourse.tile import TileContext

F32 = mybir.dt.float32
F32R = mybir.dt.float32r

N, C_IN, C_OUT, K, T, V = 16, 128, 128, 3, 256, 64
N_CORES = 8
N_PER_CORE = N // N_CORES  # 2
TC = 32                    # t-chunk size
N_CHUNKS = T // TC         # 8
QG = TC // 8               # 4 quad-groups (8 t's = 4 pairs) per chunk
OC = 2                     # chunks per output DMA (2MB transfers)
N_WARM = 24                # HAM-prewarm dummy transposes


def build(reps: int = 1):
    nc = bacc.Bacc(
        "TRN2", target_bir_lowering=False, debug=False, num_devices=N_CORES
    )
    xs = nc.dram_tensor("xs", [N_PER_CORE, C_IN, T, V], F32, kind="ExternalInput")
    wt = nc.dram_tensor("wt", [C_IN, K, C_OUT], F32, kind="ExternalInput")
    ma = nc.dram_tensor("ma", [128, 2, K, V], F32, kind="ExternalInput")
    bias2r = nc.dram_tensor("bias2r", [C_OUT, 8, V], F32, kind="ExternalInput")
    ident = nc.dram_tensor("ident", [128, 128], F32, kind="ExternalInput")
    out = nc.dram_tensor(
        "out", [N_PER_CORE, C_OUT, T, V], F32, kind="ExternalOutput"
    )

    with TileContext(nc) as tc:
        with (
            tc.tile_pool(name="const", bufs=1) as cpool,
            tc.tile_pool(name="xin", bufs=4) as xpool,
            tc.tile_pool(name="xt", bufs=3) as xtpool,
            tc.tile_pool(name="z", bufs=2) as zpool,
            tc.tile_pool(name="o", bufs=2) as opool,
            tc.tile_pool(name="ps_xt", bufs=2, space="PSUM") as ps_xt,
            tc.tile_pool(name="ps_z", bufs=2, space="PSUM") as ps_z,
            tc.tile_pool(name="ps_o", bufs=2, space="PSUM") as ps_o,
        ):
            # consts on the scalar HWDGE queue: the sync queue's first
            # x-chunk descriptor issues immediately, and ident still lands
            # within ~1.5us for the warmup matmuls
            ident_sb = cpool.tile([128, 128], F32R, tag="ident")
            nc.scalar.dma_start(out=ident_sb[:], in_=ident[:].bitcast(F32R))
            wt_sb = cpool.tile([C_IN, K, C_OUT], F32R, tag="wt")
            nc.scalar.dma_start(out=wt_sb[:], in_=wt[:].bitcast(F32R))
            ma_sb = cpool.tile([128, 2, K, V], F32R, tag="ma")
            nc.scalar.dma_start(out=ma_sb[:], in_=ma[:].bitcast(F32R))
            bias_sb = cpool.tile([C_OUT, 8, V], F32, tag="bias")
            nc.scalar.dma_start(out=bias_sb[:], in_=bias2r[:])

            # HAM prewarm: dummy ident-transposes while the first x chunk
            # is in flight, so the PE clock gate opens before real work
            warm_ps = ps_xt.tile([128, 4, 128], F32R, tag="xtp")
            for w in range(N_WARM):
                nc.tensor.matmul(
                    warm_ps[:, w % 4, :],
                    ident_sb[:],
                    ident_sb[:],
                    is_transpose=True,
                    start=True,
                    stop=True,
                    skip_group_check=True,
                )

            # Software-pipelined emission: PE's stream is in-order, so a
            # matmul that depends on a same-stage drain stalls the PE for
            # the full DVE/ACT round trip. Emit transposes of group i,
            # step A of group i-1, and step B of group i-2 so every PE op's
            # producer drain has a full group-time to land.
            for _ in range(reps):
                groups = [
                    (n, c, q)
                    for n in range(N_PER_CORE)
                    for c in range(N_CHUNKS)
                    for q in range(QG)
                ]
                st = {}   # (n, c) -> chunk state
                ost = {}  # (n, c // OC) -> output tile spanning OC chunks

                def chunk_state(n, c):
                    if (n, c) not in st:
                        x_sb = xpool.tile(
                            [C_IN, TC * V], F32R, tag="x", name="x_sb"
                        )
                        t0 = c * TC
                        nc.sync.dma_start(
                            out=x_sb[:],
                            in_=xs[n, :, t0 : t0 + TC, :].bitcast(F32R),
                        )
                        st[(n, c)] = {
                            "x": x_sb,
                            "z": zpool.tile(
                                [C_IN, TC, K, V], F32R, tag="z", name="z_sb"
                            ),
                            "xt": {},
                        }
                    return st[(n, c)]

                def out_state(n, c):
                    if (n, c // OC) not in ost:
                        ost[(n, c // OC)] = opool.tile(
                            [C_OUT, OC * TC, V], F32, tag="o", name="o_sb"
                        )
                    return ost[(n, c // OC)]

                def stage_tp(n, c, q):
                    s = chunk_state(n, c)
                    x_sb = s["x"]
                    # 4 transposes -> one PSUM bank as ONE accumulation group
                    # (start clears the bank, so only the first sets it)
                    xt_ps = ps_xt.tile([128, 4, 128], F32R, tag="xtp")
                    for j in range(4):
                        col = (4 * q + j) * 128
                        nc.tensor.matmul(
                            xt_ps[:, j, :],
                            x_sb[:, col : col + 128],
                            ident_sb[:],
                            is_transpose=True,
                            start=(j == 0),
                            stop=(j == 3),
                            skip_group_check=True,
                        )
                    xt_sb = xtpool.tile([128, 4, 128], F32R, tag="xt")
                    nc.any.tensor_copy(out=xt_sb[:], in_=xt_ps[:])
                    s["xt"][q] = xt_sb

                def stage_a(n, c, q):
                    s = chunk_state(n, c)
                    xt_sb = s["xt"].pop(q)
                    # 2 pair-matmuls into one 2-bank PSUM tile (each matmul
                    # stays inside its own 2KB bank), one batched drain
                    for h in range(2):
                        z_ps = ps_z.tile([C_IN, 2, 512], F32, tag="zp")
                        for jj in range(2):
                            nc.tensor.matmul(
                                z_ps[:, jj, 0 : 2 * K * V],
                                xt_sb[:, 2 * h + jj, :],
                                ma_sb[:],
                                start=True,
                                stop=True,
                            )
                        t0 = 8 * q + 4 * h
                        nc.any.tensor_copy(
                            out=s["z"][:, t0 : t0 + 4, :, :],
                            in_=z_ps[:, :, 0 : 2 * K * V],
                        )

                def stage_b(n, c, q):
                    s = chunk_state(n, c)
                    o_sb = out_state(n, c)
                    o_ps = ps_o.tile([C_OUT, 8, V], F32, tag="op")
                    for k in range(K):
                        nc.tensor.matmul(
                            o_ps[:],
                            wt_sb[:, k, :],
                            s["z"][:, 8 * q : 8 * (q + 1), k, :],
                            start=(k == 0),
                            stop=(k == K - 1),
                        )
                    tt = (c % OC) * TC + 8 * q
                    nc.vector.tensor_add(
                        out=o_sb[:, tt : tt + 8, :],
                        in0=o_ps[:],
                        in1=bias_sb[:],
                    )
                    if q == QG - 1:
                        del st[(n, c)]
                        if c % OC == OC - 1:
                            # output stream on the gpsimd SWDGE queue,
                            # separate from the input's sync HWDGE queue
                            c0 = (c // OC) * OC
                            nc.gpsimd.dma_start(
                                out=out[n, :, c0 * TC : (c0 + OC) * TC, :],
                                in_=o_sb[:],
                            )
                            del ost[(n, c // OC)]

                for i in range(len(groups) + 2):
                    if i < len(groups):
                        stage_tp(*groups[i])
                    if 1 <= i < len(groups) + 1:
                        stage_a(*groups[i - 1])
                    if i >= 2:
                        stage_b(*groups[i - 2])

    nc.compile()
    return nc


def prep_weights(A, W, b):
    A = np.asarray(A, np.float32)
    W = np.asarray(W, np.float32)
    b = np.asarray(b, np.float32)
    wt = np.ascontiguousarray(
        W.reshape(K, C_OUT, C_IN).transpose(2, 0, 1)
    )  # [ci, k, c]
    acat = np.ascontiguousarray(A.transpose(1, 0, 2))  # [v, k, w]
    ma = np.zeros((128, 2, K, V), np.float32)
    ma[0:64, 0] = acat
    ma[64:128, 1] = acat
    bias2 = np.einsum("kc,kw->cw", b.reshape(K, C_OUT), A.sum(axis=1))
    bias2r = np.ascontiguousarray(
        np.broadcast_to(bias2[:, None, :], (C_OUT, 8, V))
    ).astype(np.float32)
    ident = np.eye(128, dtype=np.float32)
    return wt, ma, bias2r, ident


_NC_CACHE = {}


def get_nc(reps: int = 1):
    if reps not in _NC_CACHE:
        _NC_CACHE[reps] = build(reps)
    return _NC_CACHE[reps]


def make_in_maps(x, A, W, b):
    x = np.asarray(x, np.float32)
    wt, ma, bias2r, ident = prep_weights(A, W, b)
    return [
        {
            "xs": np.ascontiguousarray(x[i * N_PER_CORE : (i + 1) * N_PER_CORE]),
            "wt": wt,
            "ma": ma,
            "bias2r": bias2r,
            "ident": ident,
        }
        for i in range(N_CORES)
    ]


def run(x, A, W, b, reps: int = 1):
    nc = get_nc(reps)
    in_maps = make_in_maps(x, A, W, b)
    res = run_bass_kernel_spmd(nc, in_maps, list(range(N_CORES)))
    return np.concatenate(
        [np.asarray(res.results[i]["out"]) for i in range(N_CORES)], axis=0
    )


def kernel(x, A, W, b):
    return run(x, A, W, b, reps=1)
